# revision 8
# baseline (speedup 1.0000x reference)
"""Trainium2 Bass kernel for nn_DetectionLoss (histogram_binning).

Computes: ce_mean + coeff * cs_mean over N=16.7M (logit-pair, label) rows,
where coeff is derived from the 2x2 confusion matrix of argmax predictions.

Strategy (data-parallel over 8 NeuronCores, N sharded along axis 0):
  Per element, with d = x1 - x0 and label l in {0,1}:
    ce_i   = softplus(d) - l*d          (== logsumexp CE for 2 classes)
    pred_i = [d > 0]                    (argmax, ties -> class 0)
    cs_i   = l * (1 - pred_i)           (M_COST[pred, l] = [pred=0 & l=1])
  Each core reduces 5 quantities to per-partition partial sums:
    S_spf = sum softplus(d)   (ACT engine, Softplus + accum_out)
    S_ld  = sum l*d           (DVE scalar_tensor_tensor + accum_out)
    TP    = sum l*[d>0]       (DVE scalar_tensor_tensor is_gt*mult + accum_out)
    P1    = sum [d>0]         (DVE tensor_scalar is_gt + accum_out)
    N1    = sum l             (GPSIMD tensor_scalar + accum_out)
  Host combines partials in float64:
    CE_sum = S_spf - S_ld;  FN = N1-TP; FP = P1-TP; TN = N-N1-P1+TP
    sens = TP/max(N1,1); prec = TP/max(P1,1)
    coeff = -0.5*log(max(sens*prec,1e-30)) if all 4 cells nonzero else 1.0
    result = CE_sum/N + coeff * FN/N
"""

import numpy as np

N_TOTAL = 16777216
N_CORES = 8
N_LOC = N_TOTAL // N_CORES  # 2097152
P = 128
F_DMA = 4096  # label elems per partition per DMA tile (outputs tile = 2*F_DMA)
F_C = 2048    # compute sub-tile free size
LAMBD = 1.0


def build_bass_kernel(n_loc=N_LOC, f_dma=F_DMA, f_c=F_C):
    """Build the per-core Bass module. Returns (nc, ncol)."""
    from contextlib import ExitStack

    import concourse.bacc as bacc
    import concourse.tile as tile
    from concourse import mybir

    assert n_loc % (P * f_dma) == 0 and f_dma % f_c == 0
    t_dma = n_loc // (P * f_dma)
    subs = f_dma // f_c
    ncol = t_dma * subs
    f32 = mybir.dt.float32
    Alu = mybir.AluOpType
    Act = mybir.ActivationFunctionType

    nc = bacc.Bacc(None)
    outs = nc.declare_dram_parameter("outputs", [n_loc, 2], f32, isOutput=False)
    labs = nc.declare_dram_parameter("labels", [n_loc], f32, isOutput=False)
    spf_o = nc.declare_dram_parameter("spf_p", [P, ncol], f32, isOutput=True)
    ld_o = nc.declare_dram_parameter("ld_p", [P, ncol], f32, isOutput=True)
    tp_o = nc.declare_dram_parameter("tp_p", [P, ncol], f32, isOutput=True)
    p1_o = nc.declare_dram_parameter("p1_p", [P, ncol], f32, isOutput=True)
    n1_o = nc.declare_dram_parameter("n1_p", [P, ncol], f32, isOutput=True)

    outs_t = outs.rearrange("(t p f) c -> t p (f c)", p=P, f=f_dma)
    labs_t = labs.rearrange("(t p f) -> t p f", p=P, f=f_dma)

    with ExitStack() as ctx:
        tc = ctx.enter_context(tile.TileContext(nc))
        dma_pool = ctx.enter_context(tc.tile_pool(name="dma", bufs=2))
        dpool = ctx.enter_context(tc.tile_pool(name="d", bufs=2))
        gpool = ctx.enter_context(tc.tile_pool(name="garbage", bufs=1))
        apool = ctx.enter_context(tc.tile_pool(name="accs", bufs=1))

        spf_a = apool.tile([P, ncol], f32, tag="spf_a")
        ld_a = apool.tile([P, ncol], f32, tag="ld_a")
        tp_a = apool.tile([P, ncol], f32, tag="tp_a")
        p1_a = apool.tile([P, ncol], f32, tag="p1_a")
        n1_a = apool.tile([P, ncol], f32, tag="n1_a")

        # Garbage destinations: accum_out is what we keep; the elementwise
        # outputs are required by the ISA but never read. One tile per
        # writing engine so no cross-engine WAW dependencies appear.
        g_e = gpool.tile([P, f_c], f32, tag="g_e")
        g_spf = gpool.tile([P, f_c], f32, tag="g_spf")
        g_ld = gpool.tile([P, f_c], f32, tag="g_ld")
        g_pl = gpool.tile([P, f_c], f32, tag="g_pl")
        g_pr = gpool.tile([P, f_c], f32, tag="g_pr")
        g_n1 = gpool.tile([P, f_c], f32, tag="g_n1")

        for t in range(t_dma):
            ot = dma_pool.tile([P, 2 * f_dma], f32, tag="ot")
            lt = dma_pool.tile([P, f_dma], f32, tag="lt")
            nc.sync.dma_start(out=ot, in_=outs_t[t])
            nc.sync.dma_start(out=lt, in_=labs_t[t])
            ot3 = ot.rearrange("p (f c) -> p f c", c=2)
            for s in range(subs):
                col = t * subs + s
                sl = slice(s * f_c, (s + 1) * f_c)
                x0 = ot3[:, sl, 0]
                x1 = ot3[:, sl, 1]
                lts = lt[:, sl]
                dt_ = dpool.tile([P, f_c], f32, tag="d")
                nc.vector.tensor_tensor(out=dt_, in0=x1, in1=x0, op=Alu.subtract)
                # softplus(d) = ln(exp(d) + 1); exp and ln share one ACT
                # table set (natural_log_exp_and_others). |d| <~ 8 so no
                # overflow concerns in fp32.
                nc.scalar.activation(out=g_e, in_=dt_, func=Act.Exp)
                nc.scalar.activation(
                    out=g_spf, in_=g_e, func=Act.Ln, bias=1.0,
                    accum_out=spf_a[:, col:col + 1],
                )
                nc.vector.scalar_tensor_tensor(
                    out=g_ld, in0=dt_, scalar=0.0, in1=lts,
                    op0=Alu.bypass, op1=Alu.mult,
                    accum_out=ld_a[:, col:col + 1],
                )
                nc.vector.scalar_tensor_tensor(
                    out=g_pl, in0=dt_, scalar=0.0, in1=lts,
                    op0=Alu.is_gt, op1=Alu.mult,
                    accum_out=tp_a[:, col:col + 1],
                )
                nc.vector.tensor_scalar(
                    out=g_pr, in0=dt_, scalar1=0.0, scalar2=None,
                    op0=Alu.is_gt, op1=Alu.add,
                    accum_out=p1_a[:, col:col + 1],
                )
                nc.scalar.activation(
                    out=g_n1, in_=lts, func=Act.Identity,
                    accum_out=n1_a[:, col:col + 1],
                )

        nc.sync.dma_start(out=spf_o[:, :], in_=spf_a)
        nc.sync.dma_start(out=ld_o[:, :], in_=ld_a)
        nc.sync.dma_start(out=tp_o[:, :], in_=tp_a)
        nc.sync.dma_start(out=p1_o[:, :], in_=p1_a)
        nc.sync.dma_start(out=n1_o[:, :], in_=n1_a)

    nc.finalize()
    return nc, ncol


def make_in_maps(outputs, labels):
    """Shard full inputs into per-core in_maps (labels cast to f32)."""
    outputs = np.asarray(outputs)
    if outputs.dtype != np.float32:
        outputs = outputs.astype(np.float32)
    labels_f = np.asarray(labels).astype(np.float32)
    in_maps = []
    for c in range(N_CORES):
        sl = slice(c * N_LOC, (c + 1) * N_LOC)
        in_maps.append({"outputs": outputs[sl], "labels": labels_f[sl]})
    return in_maps


def finish_host(per_core_results, n_total=N_TOTAL):
    """Combine per-core partial sums into the final scalar (float64 math)."""
    s_spf = s_ld = tp = p1 = n1 = 0.0
    for r in per_core_results:
        s_spf += float(np.sum(r["spf_p"], dtype=np.float64))
        s_ld += float(np.sum(r["ld_p"], dtype=np.float64))
        tp += float(np.sum(r["tp_p"], dtype=np.float64))
        p1 += float(np.sum(r["p1_p"], dtype=np.float64))
        n1 += float(np.sum(r["n1_p"], dtype=np.float64))

    ce_mean = (s_spf - s_ld) / n_total
    fn = n1 - tp
    fp = p1 - tp
    tn = n_total - n1 - p1 + tp
    all_nonzero = (tp != 0.0) and (tn != 0.0) and (fp != 0.0) and (fn != 0.0)
    sens = tp / max(tp + fn, 1.0)
    prec = tp / max(tp + fp, 1.0)
    gm_log = -0.5 * np.log(max(sens * prec, 1e-30))
    coeff = gm_log * LAMBD if all_nonzero else LAMBD
    cs_mean = fn / n_total
    return np.asarray(ce_mean + coeff * cs_mean, dtype=np.float32)


_CACHED = {}


def kernel(outputs, labels):
    from concourse.bass_utils import run_bass_kernel_spmd

    if "nc" not in _CACHED:
        _CACHED["nc"], _ = build_bass_kernel()
    nc = _CACHED["nc"]
    in_maps = make_in_maps(outputs, labels)
    res = run_bass_kernel_spmd(nc, in_maps, core_ids=list(range(N_CORES)))
    return finish_host(res.results)


# revision 10
# speedup vs baseline: 1.0243x; 1.0243x over previous
"""Trainium2 Bass kernel for nn_DetectionLoss (histogram_binning).

Computes: ce_mean + coeff * cs_mean over N=16.7M (logit-pair, label) rows,
where coeff is derived from the 2x2 confusion matrix of argmax predictions.

Strategy (data-parallel over 8 NeuronCores, N sharded along axis 0):
  Per element, with d = x1 - x0 and label l in {0,1}:
    ce_i   = softplus(d) - l*d          (== logsumexp CE for 2 classes)
    pred_i = [d > 0]                    (argmax, ties -> class 0)
    cs_i   = l * (1 - pred_i)           (M_COST[pred, l] = [pred=0 & l=1])
  Each core reduces 5 quantities to per-partition partial sums:
    S_spf = sum softplus(d)   (ACT engine, Softplus + accum_out)
    S_ld  = sum l*d           (DVE scalar_tensor_tensor + accum_out)
    TP    = sum l*[d>0]       (DVE scalar_tensor_tensor is_gt*mult + accum_out)
    P1    = sum [d>0]         (DVE tensor_scalar is_gt + accum_out)
    N1    = sum l             (GPSIMD tensor_scalar + accum_out)
  Host combines partials in float64:
    CE_sum = S_spf - S_ld;  FN = N1-TP; FP = P1-TP; TN = N-N1-P1+TP
    sens = TP/max(N1,1); prec = TP/max(P1,1)
    coeff = -0.5*log(max(sens*prec,1e-30)) if all 4 cells nonzero else 1.0
    result = CE_sum/N + coeff * FN/N
"""

import numpy as np

N_TOTAL = 16777216
N_CORES = 8
N_LOC = N_TOTAL // N_CORES  # 2097152
P = 128
F_DMA = 4096  # label elems per partition per DMA tile (outputs tile = 2*F_DMA)
F_C = 2048    # compute sub-tile free size
LAMBD = 1.0


def build_bass_kernel(n_loc=N_LOC, f_dma=F_DMA, f_c=F_C, d_on_gpsimd=True):
    """Build the per-core Bass module. Returns (nc, ncol).

    Engine split (per DMA tile of [128, f_dma] label elems):
      GPSIMD: d = x1 - x0 (one TT over the whole tile)
      ACT:    exp(d) -> g_e; ln(g_e + 1) in-place, accum -> sum softplus
      DVE:    per f_c sub-tile: l*d (stt, accum), [l*d>0] (TS 2x, accum=TP),
              [d>0] (TS 2x, accum=p1)
      PE:     n1 = sum(l) via ones-matmul into PSUM accumulator
    """
    from contextlib import ExitStack

    import concourse.bacc as bacc
    import concourse.tile as tile
    from concourse import mybir

    assert n_loc % (P * f_dma) == 0 and f_dma % f_c == 0
    t_dma = n_loc // (P * f_dma)
    subs = f_dma // f_c
    ncol = t_dma * subs
    f32 = mybir.dt.float32
    Alu = mybir.AluOpType
    Act = mybir.ActivationFunctionType
    NMM = min(512, f_dma)  # matmul moving free-dim (one PSUM bank of fp32)

    nc = bacc.Bacc(None)
    outs = nc.declare_dram_parameter("outputs", [n_loc, 2], f32, isOutput=False)
    labs = nc.declare_dram_parameter("labels", [n_loc], f32, isOutput=False)
    spf_o = nc.declare_dram_parameter("spf_p", [P, t_dma], f32, isOutput=True)
    ld_o = nc.declare_dram_parameter("ld_p", [P, ncol], f32, isOutput=True)
    tp_o = nc.declare_dram_parameter("tp_p", [P, ncol], f32, isOutput=True)
    p1_o = nc.declare_dram_parameter("p1_p", [P, ncol], f32, isOutput=True)
    n1_o = nc.declare_dram_parameter("n1_p", [1, NMM], f32, isOutput=True)

    outs_t = outs.rearrange("(t p f) c -> t p (f c)", p=P, f=f_dma)
    labs_t = labs.rearrange("(t p f) -> t p f", p=P, f=f_dma)
    n_mm = f_dma // NMM

    with ExitStack() as ctx:
        tc = ctx.enter_context(tile.TileContext(nc))
        dma_pool = ctx.enter_context(tc.tile_pool(name="dma", bufs=2))
        dpool = ctx.enter_context(tc.tile_pool(name="d", bufs=2))
        gpool = ctx.enter_context(tc.tile_pool(name="garbage", bufs=1))
        apool = ctx.enter_context(tc.tile_pool(name="accs", bufs=1))
        pspool = ctx.enter_context(tc.tile_pool(name="ps", bufs=1, space="PSUM"))

        spf_a = apool.tile([P, t_dma], f32, tag="spf_a")
        ld_a = apool.tile([P, ncol], f32, tag="ld_a")
        tp_a = apool.tile([P, ncol], f32, tag="tp_a")
        p1_a = apool.tile([P, ncol], f32, tag="p1_a")

        ones = apool.tile([P, 1], f32, tag="ones")
        nc.vector.memset(ones, 1.0)
        n1_ps = pspool.tile([1, NMM], f32, tag="n1_ps")

        # Garbage destinations: accum_out is what we keep; the elementwise
        # outputs are required by the ISA but only g_e/g_ld are re-read
        # (by the same engine that wrote them).
        g_e = gpool.tile([P, f_dma], f32, tag="g_e")
        g_ld = gpool.tile([P, f_c], f32, tag="g_ld")
        g_tp = gpool.tile([P, f_c], f32, tag="g_tp")
        g_pr = gpool.tile([P, f_c], f32, tag="g_pr")

        mm_i = 0
        for t in range(t_dma):
            ot = dma_pool.tile([P, 2 * f_dma], f32, tag="ot")
            lt = dma_pool.tile([P, f_dma], f32, tag="lt")
            nc.sync.dma_start(out=ot, in_=outs_t[t])
            nc.sync.dma_start(out=lt, in_=labs_t[t])
            ot3 = ot.rearrange("p (f c) -> p f c", c=2)
            x0 = ot3[:, :, 0]
            x1 = ot3[:, :, 1]
            dt_ = dpool.tile([P, f_dma], f32, tag="d")
            if d_on_gpsimd:
                nc.gpsimd.tensor_tensor(out=dt_, in0=x1, in1=x0, op=Alu.subtract)
            else:
                nc.vector.tensor_tensor(out=dt_, in0=x1, in1=x0, op=Alu.subtract)
            # softplus(d) = ln(exp(d) + 1); exp/ln/identity share the
            # natural_log_exp_and_others ACT table set. |d| <~ 9 in this
            # problem so exp(d) stays well inside fp32 range.
            nc.scalar.activation(out=g_e, in_=dt_, func=Act.Exp)
            nc.scalar.activation(
                out=g_e, in_=g_e, func=Act.Ln, bias=1.0,
                accum_out=spf_a[:, t:t + 1],
            )
            for s in range(subs):
                col = t * subs + s
                sl = slice(s * f_c, (s + 1) * f_c)
                nc.vector.scalar_tensor_tensor(
                    out=g_ld, in0=dt_[:, sl], scalar=0.0, in1=lt[:, sl],
                    op0=Alu.bypass, op1=Alu.mult,
                    accum_out=ld_a[:, col:col + 1],
                )
                nc.vector.tensor_scalar(
                    out=g_tp, in0=g_ld, scalar1=0.0, scalar2=None,
                    op0=Alu.is_gt, op1=Alu.add,
                    accum_out=tp_a[:, col:col + 1],
                )
                nc.vector.tensor_scalar(
                    out=g_pr, in0=dt_[:, sl], scalar1=0.0, scalar2=None,
                    op0=Alu.is_gt, op1=Alu.add,
                    accum_out=p1_a[:, col:col + 1],
                )
            for m in range(n_mm):
                nc.tensor.matmul(
                    n1_ps[:, :], lhsT=ones, rhs=lt[:, m * NMM:(m + 1) * NMM],
                    start=(mm_i == 0), stop=(mm_i == t_dma * n_mm - 1),
                )
                mm_i += 1

        n1_sb = apool.tile([1, NMM], f32, tag="n1_sb")
        nc.vector.tensor_copy(out=n1_sb, in_=n1_ps)
        nc.sync.dma_start(out=spf_o[:, :], in_=spf_a)
        nc.sync.dma_start(out=ld_o[:, :], in_=ld_a)
        nc.sync.dma_start(out=tp_o[:, :], in_=tp_a)
        nc.sync.dma_start(out=p1_o[:, :], in_=p1_a)
        nc.sync.dma_start(out=n1_o[:, :], in_=n1_sb)

    nc.finalize()
    return nc, ncol


def make_in_maps(outputs, labels):
    """Shard full inputs into per-core in_maps (labels cast to f32)."""
    outputs = np.asarray(outputs)
    if outputs.dtype != np.float32:
        outputs = outputs.astype(np.float32)
    labels_f = np.asarray(labels).astype(np.float32)
    in_maps = []
    for c in range(N_CORES):
        sl = slice(c * N_LOC, (c + 1) * N_LOC)
        in_maps.append({"outputs": outputs[sl], "labels": labels_f[sl]})
    return in_maps


def finish_host(per_core_results, n_total=N_TOTAL):
    """Combine per-core partial sums into the final scalar (float64 math)."""
    s_spf = s_ld = tp = p1 = n1 = 0.0
    for r in per_core_results:
        s_spf += float(np.sum(r["spf_p"], dtype=np.float64))
        s_ld += float(np.sum(r["ld_p"], dtype=np.float64))
        tp += float(np.sum(r["tp_p"], dtype=np.float64))
        p1 += float(np.sum(r["p1_p"], dtype=np.float64))
        n1 += float(np.sum(r["n1_p"], dtype=np.float64))

    ce_mean = (s_spf - s_ld) / n_total
    fn = n1 - tp
    fp = p1 - tp
    tn = n_total - n1 - p1 + tp
    all_nonzero = (tp != 0.0) and (tn != 0.0) and (fp != 0.0) and (fn != 0.0)
    sens = tp / max(tp + fn, 1.0)
    prec = tp / max(tp + fp, 1.0)
    gm_log = -0.5 * np.log(max(sens * prec, 1e-30))
    coeff = gm_log * LAMBD if all_nonzero else LAMBD
    cs_mean = fn / n_total
    return np.asarray(ce_mean + coeff * cs_mean, dtype=np.float32)


_CACHED = {}


def kernel(outputs, labels):
    from concourse.bass_utils import run_bass_kernel_spmd

    if "nc" not in _CACHED:
        _CACHED["nc"], _ = build_bass_kernel()
    nc = _CACHED["nc"]
    in_maps = make_in_maps(outputs, labels)
    res = run_bass_kernel_spmd(nc, in_maps, core_ids=list(range(N_CORES)))
    return finish_host(res.results)
